# revision 3
# baseline (speedup 1.0000x reference)
"""Trainium2 Bass kernel for nn_Encoder_Postnet_combine (B=16,T=4096,P=512,D=512,S=100).

Math (algebraically folded from the reference):
  idx[b,t]   : sequential aligner scan (host, tiny integer recurrence)
  W1 = w_out[:D]; W2 = w_out[D:]
  Wc  = (I + w_pos) @ W1
  EW  = encoder_out @ Wc                       (device GEMM, per batch)
  v   = w_pitch[0] @ W1
  dEb = (emb_beats[1]-emb_beats[0]) @ W1
  EsW = emb_singer @ W2
  PEW = pe @ (w_pos @ W1) + (b_pitch+b_pos+emb_beats[0]) @ W1 + b_out
  out = leaky( EW[b,idx] + EsW[sv] + PEW[t] + pitch*v + beats*dEb , 0.01)

Device mapping (bf16 matmul inputs, fp32 PSUM accumulation):
  idx is monotone nondecreasing (steps of <=1), so each 128-frame output
  tile reads a <=128-row window of EW spanning <=2 of the 4 per-batch EW
  SBUF tiles.  The gather is a one-hot matmul accumulated in PSUM:
    ps  = sum_j oh1_j.T @ EW_tile_j   (aligner gather, per-tile j set baked)
        + ohsp.T @ esb                (singer one-hot rows 0..99, row 100 =
                                       pitch values, row 101 = beats values;
                                       esb rows = [EsW; v; dEb])
        + I.T @ pew_tile              (positional table add)
  out = ACT lrelu(ps)  -> DMA store.
The per-tile segment structure (union across cores, SPMD) is baked into the
program; one-hot contents ship as runtime tensors (zero blocks where a core
does not need a segment).

Sharding: data-parallel over batch, 2 batches per core on 8 cores.
"""
import numpy as np
import ml_dtypes

import concourse.bass as bass
import concourse.mybir as mybir
import concourse.tile as tile
from concourse.vector_clock import ScopedClock
from concourse.bass_utils import run_bass_kernel_spmd

F32 = mybir.dt.float32
BF16 = mybir.dt.bfloat16
BF16_NP = ml_dtypes.bfloat16

B, T, PH, D, S = 16, 4096, 512, 512, 100
NCORES = 8
BPC = B // NCORES          # batches per core
TT = T // 128              # 32 t-tiles per batch
NT = BPC * TT              # 64 tiles per core
NEW = PH // 128            # 4 EW tiles per batch

# ---------------------------------------------------------------------------
# Workarounds for this walrus build: at most ONE sync wait per instruction
# (EventSemaphore: 2).


def _split_drain_and_barrier(self, tick_clock, wait_clock):
    nc = self.nc
    probe = nc.sync.nop()
    wait_clock.add_sem_waits(probe.ins, ScopedClock({None: tick_clock.global_clock}))
    si = probe.ins.sync_info
    if si is not None and si.on_wait and len(si.on_wait) > 1:
        waits = list(si.on_wait)
        si.on_wait = waits[:1]
        for w in waits[1:]:
            extra = nc.sync.nop()
            extra.ins.sync_info = mybir.SyncInfo(on_wait=[w], on_update=[])
    nc.sync.drain()
    nc.all_engine_barrier()
    assert self.sems is not None
    popped = nc._tile_sem_poison_stack.pop()
    assert popped is self._sem_poison
    nc.clear_and_free_semaphores(list(self.sems.allocated().values()))
    nc.all_engine_barrier()


tile.TileContext._drain_and_barrier = _split_drain_and_barrier


def _split_multi_waits(nc):
    counter = [0]

    def fresh_nop(engine, wait):
        counter[0] += 1
        nop = mybir.InstNoOp(name=f"waitsplit_{counter[0]}", ins=[], outs=[])
        nop.engine = engine
        nop.sync_info = mybir.SyncInfo(on_wait=[wait], on_update=[])
        return nop

    for fn in nc.m.functions:
        for blk in fn.blocks:
            new_insts = []
            for inst in blk.instructions:
                si = inst.sync_info
                limit = 2 if isinstance(inst, mybir.InstEventSemaphore) else 1
                if si is not None and si.on_wait and len(si.on_wait) > limit:
                    waits = list(si.on_wait)
                    for w in waits[:-limit]:
                        new_insts.append(fresh_nop(inst.engine, w))
                    si.on_wait = waits[-limit:]
                new_insts.append(inst)
            blk.instructions = new_insts


# ---------------------------------------------------------------------------
# Device program.  `tile_segs` is a length-NT list; tile_segs[k] is the list
# of (ew_tile_index, oh1_col_block) pairs baked for that output tile.


def build_program(tile_segs, nseg, repeat=1):
    nc = bass.Bass()
    encT = nc.declare_dram_parameter("encT", [BPC * PH, D], BF16, isOutput=False)
    wc = nc.declare_dram_parameter("wc", [D, D], BF16, isOutput=False)
    pew = nc.declare_dram_parameter("pew", [T, D], BF16, isOutput=False)
    esb = nc.declare_dram_parameter("esb", [128, D], BF16, isOutput=False)
    iden = nc.declare_dram_parameter("iden", [128, 128], BF16, isOutput=False)
    oh1 = nc.declare_dram_parameter("oh1", [128, nseg * 128], BF16, isOutput=False)
    ohsp = nc.declare_dram_parameter("ohsp", [128, NT * 128], BF16, isOutput=False)
    out = nc.declare_dram_parameter("out", [BPC * T, D], F32, isOutput=True)

    with tile.TileContext(nc) as tc:
        with (
            tc.tile_pool(name="const", bufs=1) as cpool,
            tc.tile_pool(name="pew", bufs=4) as ppool,
            tc.tile_pool(name="outp", bufs=6) as opool,
            tc.tile_pool(name="psA", bufs=2, space="PSUM") as psumA,
            tc.tile_pool(name="psB", bufs=6, space="PSUM") as psumB,
        ):
            def body(_=None):
                # --- small/const inputs (phase-A deps first on the ring) ---
                wc_sb = []
                for ki in range(4):
                    w_t = cpool.tile([128, D], BF16, tag=f"wc{ki}")
                    nc.sync.dma_start(out=w_t[:], in_=wc[ki * 128:(ki + 1) * 128, :])
                    wc_sb.append(w_t)
                encT_sb = []
                for j in range(4 * BPC):
                    e_t = cpool.tile([128, D], BF16, tag=f"encT{j}")
                    nc.sync.dma_start(out=e_t[:], in_=encT[j * 128:(j + 1) * 128, :])
                    encT_sb.append(e_t)
                esb_sb = cpool.tile([128, D], BF16, tag="esb")
                nc.sync.dma_start(out=esb_sb[:], in_=esb[:])
                id_sb = cpool.tile([128, 128], BF16, tag="iden")
                nc.sync.dma_start(out=id_sb[:], in_=iden[:])
                oh1_sb = cpool.tile([128, nseg * 128], BF16, tag="oh1")
                nc.sync.dma_start(out=oh1_sb[:], in_=oh1[:])
                ohsp_sb = cpool.tile([128, NT * 128], BF16, tag="ohsp")
                nc.sync.dma_start(out=ohsp_sb[:], in_=ohsp[:])

                # --- phase A: EW = E @ Wc (per batch), cast to bf16 in SBUF ---
                ew_sb = []
                for b in range(BPC):
                    for mm in range(4):
                        ps = psumA.tile([128, D], F32, tag="ps_ew")
                        for ki in range(4):
                            nc.tensor.matmul(
                                out=ps[:],
                                lhsT=encT_sb[b * 4 + ki][:, mm * 128:(mm + 1) * 128],
                                rhs=wc_sb[ki][:],
                                start=(ki == 0),
                                stop=(ki == 3),
                            )
                        ew_t = cpool.tile([128, D], BF16, tag=f"ew{b}_{mm}")
                        nc.vector.tensor_copy(out=ew_t[:], in_=ps[:])
                        ew_sb.append(ew_t)

                # --- phase B: one-hot gathers + PEW, all PSUM-accumulated ---
                for tt in range(TT):
                    pew_t = ppool.tile([128, D], BF16, tag="pew_t")
                    nc.sync.dma_start(out=pew_t[:], in_=pew[tt * 128:(tt + 1) * 128, :])
                    for b in range(BPC):
                        k = tt * BPC + b
                        segs = tile_segs[k]
                        ps = psumB.tile([128, D], F32, tag="ps_b")
                        for si, (ew_i, col) in enumerate(segs):
                            nc.tensor.matmul(
                                out=ps[:],
                                lhsT=oh1_sb[:, col * 128:(col + 1) * 128],
                                rhs=ew_sb[ew_i][:],
                                start=(si == 0),
                                stop=False,
                            )
                        nc.tensor.matmul(
                            out=ps[:],
                            lhsT=ohsp_sb[:, k * 128:(k + 1) * 128],
                            rhs=esb_sb[:],
                            start=False,
                            stop=False,
                        )
                        nc.tensor.matmul(
                            out=ps[:],
                            lhsT=id_sb[:],
                            rhs=pew_t[:],
                            start=False,
                            stop=True,
                        )
                        o_t = opool.tile([128, D], F32, tag="o_t")
                        nc.scalar.activation(out=o_t[:], in_=ps[:],
                                             func=mybir.ActivationFunctionType.Lrelu,
                                             alpha=0.01)
                        r0 = b * T + tt * 128
                        nc.scalar.dma_start(out=out[r0:r0 + 128, :], in_=o_t[:])

            for _ in range(repeat):
                body()

    _split_multi_waits(nc)
    return nc


# ---------------------------------------------------------------------------
# Host side


def _host_scan_idx(align, text):
    align = np.asarray(align, dtype=np.int64)
    text = np.asarray(text, dtype=np.int64)
    Bn, Tn = align.shape
    Pn = text.shape[1]
    idx = np.zeros((Bn, Tn), dtype=np.int32)
    ind = np.zeros(Bn, dtype=np.int64)
    rows = np.arange(Bn)
    cur = text[rows, ind]
    for t in range(1, Tn):
        a = align[:, t]
        stay = a == cur
        ind = np.where(stay, ind, np.minimum(ind + 1, Pn - 1))
        cur = np.where(stay, cur, text[rows, ind])
        idx[:, t] = ind
    return idx


def _positional_encoding(length, d_model):
    pos = np.arange(length, dtype=np.float32)[:, None]
    div = np.exp(np.arange(0, d_model, 2, dtype=np.float32)
                 * (-np.log(10000.0) / d_model))
    pe = np.zeros((length, d_model), np.float32)
    pe[:, 0::2] = np.sin(pos * div)
    pe[:, 1::2] = np.cos(pos * div)
    return pe


def _fold(w_pitch, b_pitch, w_pos, b_pos, emb_beats, emb_singer, w_out, b_out):
    f64 = np.float64
    W1 = np.asarray(w_out[:D], f64)
    W2 = np.asarray(w_out[D:], f64)
    WposW1 = np.asarray(w_pos, f64) @ W1
    Wc = (W1 + WposW1).astype(np.float32)
    v = (np.asarray(w_pitch[0], f64) @ W1).astype(np.float32)
    EbW = np.asarray(emb_beats, f64) @ W1
    dEb = (EbW[1] - EbW[0]).astype(np.float32)
    EsW = (np.asarray(emb_singer, f64) @ W2).astype(np.float32)
    cb = (np.asarray(b_pitch + b_pos, f64) @ W1 + EbW[0] + np.asarray(b_out, f64))
    pe = _positional_encoding(T, D)
    PEW = (np.asarray(pe, f64) @ WposW1 + cb[None, :]).astype(np.float32)
    return Wc, v, dEb, EsW, PEW


def _tile_blocks(idx):
    """Per-core needed EW-block sets: blocks[c][k] = sorted j list for that
    core's output tile k (j indexes the 4 per-batch phone blocks)."""
    blocks = []
    for c in range(NCORES):
        per_tile = []
        for tt in range(TT):
            for b in range(BPC):
                row = idx[c * BPC + b, tt * 128:(tt + 1) * 128]
                per_tile.append(sorted(set(int(x) // 128 for x in (row[0], row[-1]))))
        blocks.append(per_tile)
    return blocks


_CACHE = {}


def kernel(encoder_out, align_phone, text_phone, pitch, beats, singer_vec,
           w_pitch, b_pitch, w_pos, b_pos, emb_beats, emb_singer, w_out, b_out):
    encoder_out = np.ascontiguousarray(np.asarray(encoder_out, np.float32))
    pitch = np.asarray(pitch, np.float32)[..., 0]          # [B,T]
    beats_f = np.asarray(beats, np.int64)[..., 0].astype(np.float32)
    sv = np.asarray(singer_vec, np.int64)[..., 0].astype(np.int64)  # [B,T]

    idx = _host_scan_idx(align_phone, text_phone)          # [B,T] int32
    Wc, v, dEb, EsW, PEW = _fold(
        np.asarray(w_pitch, np.float32), np.asarray(b_pitch, np.float32),
        np.asarray(w_pos, np.float32), np.asarray(b_pos, np.float32),
        np.asarray(emb_beats, np.float32), np.asarray(emb_singer, np.float32),
        np.asarray(w_out, np.float32), np.asarray(b_out, np.float32))

    esb = np.zeros((128, D), np.float32)
    esb[:S] = EsW
    esb[100] = v
    esb[101] = dEb

    # Baked structure: per tile, union over cores of needed EW blocks.
    blocks = _tile_blocks(idx)
    tile_segs = []
    seg_cols = {}                     # (k, j) -> oh1 column block
    col = 0
    for k in range(NT):
        b = k % BPC
        union_j = sorted(set(j for c in range(NCORES) for j in blocks[c][k]))
        segs = []
        for j in union_j:
            seg_cols[(k, j)] = col
            segs.append((b * NEW + j, col))
            col += 1
        tile_segs.append(segs)
    nseg = col

    tpos = np.arange(128)
    in_maps = []
    for c in range(NCORES):
        b0 = c * BPC
        sl = slice(b0, b0 + BPC)
        encT = np.ascontiguousarray(
            encoder_out[sl].transpose(0, 2, 1).reshape(BPC * PH, D))
        oh1 = np.zeros((128, nseg * 128), np.float32)
        ohsp = np.zeros((128, NT * 128), np.float32)
        for k in range(NT):
            tt, b = k // BPC, k % BPC
            t0 = tt * 128
            idxs = idx[b0 + b, t0:t0 + 128].astype(np.int64)
            for j in blocks[c][k]:
                local = idxs - j * 128
                m = (local >= 0) & (local < 128)
                blk = oh1[:, seg_cols[(k, j)] * 128:(seg_cols[(k, j)] + 1) * 128]
                blk[local[m], tpos[m]] = 1.0
            blk = ohsp[:, k * 128:(k + 1) * 128]
            blk[sv[b0 + b, t0:t0 + 128], tpos] = 1.0
            blk[100, :] = pitch[b0 + b, t0:t0 + 128]
            blk[101, :] = beats_f[b0 + b, t0:t0 + 128]
        in_maps.append({
            "encT": encT.astype(BF16_NP),
            "wc": Wc.astype(BF16_NP),
            "pew": PEW.astype(BF16_NP),
            "esb": esb.astype(BF16_NP),
            "iden": np.eye(128, dtype=BF16_NP),
            "oh1": oh1.astype(BF16_NP),
            "ohsp": ohsp.astype(BF16_NP),
        })

    seg_key = tuple(tuple(s) for s in tile_segs)
    if _CACHE.get("seg_key") != seg_key:
        _CACHE["nc"] = build_program(tile_segs, nseg)
        _CACHE["seg_key"] = seg_key
        _CACHE["segs"] = (tile_segs, nseg)
    nc = _CACHE["nc"]

    _CACHE["last_in_maps"] = in_maps
    res = run_bass_kernel_spmd(nc, in_maps, core_ids=list(range(NCORES)))
    out = np.empty((B, T, D), np.float32)
    for c in range(NCORES):
        out[c * BPC:(c + 1) * BPC] = res.results[c]["out"].reshape(BPC, T, D)
    return out


# revision 5
# speedup vs baseline: 1.2552x; 1.2552x over previous
"""Trainium2 Bass kernel for nn_Encoder_Postnet_combine (B=16,T=4096,P=512,D=512,S=100).

Math (algebraically folded from the reference):
  idx[b,t]   : sequential aligner scan (host, tiny integer recurrence)
  W1 = w_out[:D]; W2 = w_out[D:]
  Wc  = (I + w_pos) @ W1
  EW  = encoder_out @ Wc                       (device GEMM, per batch)
  v   = w_pitch[0] @ W1
  dEb = (emb_beats[1]-emb_beats[0]) @ W1
  EsW = emb_singer @ W2
  PEW = pe @ (w_pos @ W1) + (b_pitch+b_pos+emb_beats[0]) @ W1 + b_out
  out = leaky( EW[b,idx] + EsW[sv] + PEW[t] + pitch*v + beats*dEb , 0.01)

Device mapping (bf16 matmul inputs, fp32 PSUM accumulation):
  idx is monotone nondecreasing (steps of <=1), so each 128-frame output
  tile reads a <=128-row window of EW spanning <=2 of the 4 per-batch EW
  SBUF tiles.  The gather is a one-hot matmul accumulated in PSUM:
    ps  = sum_j oh1_j.T @ EW_tile_j   (aligner gather, per-tile j set baked)
        + ohsp.T @ esb                (singer one-hot rows 0..99, row 100 =
                                       pitch values, row 101 = beats values;
                                       esb rows = [EsW; v; dEb])
        + I.T @ pew_tile              (positional table add)
  out = ACT lrelu(ps)  -> DMA store.
The per-tile segment structure (union across cores, SPMD) is baked into the
program; one-hot contents ship as runtime tensors (zero blocks where a core
does not need a segment).

Sharding: data-parallel over batch, 2 batches per core on 8 cores.
"""
import numpy as np
import ml_dtypes

import concourse.bass as bass
import concourse.mybir as mybir
import concourse.tile as tile
from concourse.vector_clock import ScopedClock
from concourse.bass_utils import run_bass_kernel_spmd

F32 = mybir.dt.float32
BF16 = mybir.dt.bfloat16
BF16_NP = ml_dtypes.bfloat16

# Device-side output dtype: bf16 halves the dominant HBM store traffic; the
# host upcasts to fp32.  Worst-case added error ~0.4% of |out| — well inside
# the 2e-2 rel tolerance.
OUT_DT = BF16
OUT_NP = BF16_NP

B, T, PH, D, S = 16, 4096, 512, 512, 100
NCORES = 8
BPC = B // NCORES          # batches per core
TT = T // 128              # 32 t-tiles per batch
NT = BPC * TT              # 64 tiles per core
NEW = PH // 128            # 4 EW tiles per batch

# ---------------------------------------------------------------------------
# Workarounds for this walrus build: at most ONE sync wait per instruction
# (EventSemaphore: 2).


def _split_drain_and_barrier(self, tick_clock, wait_clock):
    nc = self.nc
    probe = nc.sync.nop()
    wait_clock.add_sem_waits(probe.ins, ScopedClock({None: tick_clock.global_clock}))
    si = probe.ins.sync_info
    if si is not None and si.on_wait and len(si.on_wait) > 1:
        waits = list(si.on_wait)
        si.on_wait = waits[:1]
        for w in waits[1:]:
            extra = nc.sync.nop()
            extra.ins.sync_info = mybir.SyncInfo(on_wait=[w], on_update=[])
    nc.sync.drain()
    nc.all_engine_barrier()
    assert self.sems is not None
    popped = nc._tile_sem_poison_stack.pop()
    assert popped is self._sem_poison
    nc.clear_and_free_semaphores(list(self.sems.allocated().values()))
    nc.all_engine_barrier()


tile.TileContext._drain_and_barrier = _split_drain_and_barrier


def _split_multi_waits(nc):
    counter = [0]

    def fresh_nop(engine, wait):
        counter[0] += 1
        nop = mybir.InstNoOp(name=f"waitsplit_{counter[0]}", ins=[], outs=[])
        nop.engine = engine
        nop.sync_info = mybir.SyncInfo(on_wait=[wait], on_update=[])
        return nop

    for fn in nc.m.functions:
        for blk in fn.blocks:
            new_insts = []
            for inst in blk.instructions:
                si = inst.sync_info
                limit = 2 if isinstance(inst, mybir.InstEventSemaphore) else 1
                if si is not None and si.on_wait and len(si.on_wait) > limit:
                    waits = list(si.on_wait)
                    for w in waits[:-limit]:
                        new_insts.append(fresh_nop(inst.engine, w))
                    si.on_wait = waits[-limit:]
                new_insts.append(inst)
            blk.instructions = new_insts


# ---------------------------------------------------------------------------
# Device program.  `tile_segs` is a length-NT list; tile_segs[k] is the list
# of (ew_tile_index, oh1_col_block) pairs baked for that output tile.


def build_program(tile_segs, nseg, repeat=1):
    nc = bass.Bass()
    encT = nc.declare_dram_parameter("encT", [BPC * PH, D], BF16, isOutput=False)
    wc = nc.declare_dram_parameter("wc", [D, D], BF16, isOutput=False)
    pew = nc.declare_dram_parameter("pew", [T, D], BF16, isOutput=False)
    esb = nc.declare_dram_parameter("esb", [128, D], BF16, isOutput=False)
    iden = nc.declare_dram_parameter("iden", [128, 128], BF16, isOutput=False)
    oh1 = nc.declare_dram_parameter("oh1", [128, nseg * 128], BF16, isOutput=False)
    ohsp = nc.declare_dram_parameter("ohsp", [128, NT * 128], BF16, isOutput=False)
    out = nc.declare_dram_parameter("out", [BPC * T, D], OUT_DT, isOutput=True)

    with tile.TileContext(nc) as tc:
        with (
            tc.tile_pool(name="const", bufs=1) as cpool,
            tc.tile_pool(name="pew", bufs=4) as ppool,
            tc.tile_pool(name="outp", bufs=6) as opool,
            tc.tile_pool(name="psA", bufs=2, space="PSUM") as psumA,
            tc.tile_pool(name="psB", bufs=6, space="PSUM") as psumB,
        ):
            def body(_=None):
                # --- small/const inputs (phase-A deps first on the ring) ---
                wc_sb = []
                for ki in range(4):
                    w_t = cpool.tile([128, D], BF16, tag=f"wc{ki}")
                    nc.sync.dma_start(out=w_t[:], in_=wc[ki * 128:(ki + 1) * 128, :])
                    wc_sb.append(w_t)
                encT_sb = []
                for j in range(4 * BPC):
                    e_t = cpool.tile([128, D], BF16, tag=f"encT{j}")
                    nc.sync.dma_start(out=e_t[:], in_=encT[j * 128:(j + 1) * 128, :])
                    encT_sb.append(e_t)
                esb_sb = cpool.tile([128, D], BF16, tag="esb")
                nc.sync.dma_start(out=esb_sb[:], in_=esb[:])
                id_sb = cpool.tile([128, 128], BF16, tag="iden")
                nc.sync.dma_start(out=id_sb[:], in_=iden[:])
                oh1_sb = cpool.tile([128, nseg * 128], BF16, tag="oh1")
                nc.sync.dma_start(out=oh1_sb[:], in_=oh1[:])
                ohsp_sb = cpool.tile([128, NT * 128], BF16, tag="ohsp")
                nc.sync.dma_start(out=ohsp_sb[:], in_=ohsp[:])

                # --- phase A: EW = E @ Wc (per batch), cast to bf16 in SBUF ---
                ew_sb = []
                for b in range(BPC):
                    for mm in range(4):
                        ps = psumA.tile([128, D], F32, tag="ps_ew")
                        for ki in range(4):
                            nc.tensor.matmul(
                                out=ps[:],
                                lhsT=encT_sb[b * 4 + ki][:, mm * 128:(mm + 1) * 128],
                                rhs=wc_sb[ki][:],
                                start=(ki == 0),
                                stop=(ki == 3),
                            )
                        ew_t = cpool.tile([128, D], BF16, tag=f"ew{b}_{mm}")
                        nc.vector.tensor_copy(out=ew_t[:], in_=ps[:])
                        ew_sb.append(ew_t)

                # --- phase B: one-hot gathers + PEW, all PSUM-accumulated ---
                for tt in range(TT):
                    pew_t = ppool.tile([128, D], BF16, tag="pew_t")
                    nc.sync.dma_start(out=pew_t[:], in_=pew[tt * 128:(tt + 1) * 128, :])
                    for b in range(BPC):
                        k = tt * BPC + b
                        segs = tile_segs[k]
                        ps = psumB.tile([128, D], F32, tag="ps_b")
                        for si, (ew_i, col) in enumerate(segs):
                            nc.tensor.matmul(
                                out=ps[:],
                                lhsT=oh1_sb[:, col * 128:(col + 1) * 128],
                                rhs=ew_sb[ew_i][:],
                                start=(si == 0),
                                stop=False,
                            )
                        nc.tensor.matmul(
                            out=ps[:],
                            lhsT=ohsp_sb[:, k * 128:(k + 1) * 128],
                            rhs=esb_sb[:],
                            start=False,
                            stop=False,
                        )
                        nc.tensor.matmul(
                            out=ps[:],
                            lhsT=id_sb[:],
                            rhs=pew_t[:],
                            start=False,
                            stop=True,
                        )
                        o_t = opool.tile([128, D], OUT_DT, tag="o_t")
                        nc.scalar.activation(out=o_t[:], in_=ps[:],
                                             func=mybir.ActivationFunctionType.Lrelu,
                                             alpha=0.01)
                        r0 = b * T + tt * 128
                        nc.scalar.dma_start(out=out[r0:r0 + 128, :], in_=o_t[:])

            for _ in range(repeat):
                body()

    _split_multi_waits(nc)
    return nc


# ---------------------------------------------------------------------------
# Host side


def _host_scan_idx(align, text):
    align = np.asarray(align, dtype=np.int64)
    text = np.asarray(text, dtype=np.int64)
    Bn, Tn = align.shape
    Pn = text.shape[1]
    idx = np.zeros((Bn, Tn), dtype=np.int32)
    ind = np.zeros(Bn, dtype=np.int64)
    rows = np.arange(Bn)
    cur = text[rows, ind]
    for t in range(1, Tn):
        a = align[:, t]
        stay = a == cur
        ind = np.where(stay, ind, np.minimum(ind + 1, Pn - 1))
        cur = np.where(stay, cur, text[rows, ind])
        idx[:, t] = ind
    return idx


def _positional_encoding(length, d_model):
    pos = np.arange(length, dtype=np.float32)[:, None]
    div = np.exp(np.arange(0, d_model, 2, dtype=np.float32)
                 * (-np.log(10000.0) / d_model))
    pe = np.zeros((length, d_model), np.float32)
    pe[:, 0::2] = np.sin(pos * div)
    pe[:, 1::2] = np.cos(pos * div)
    return pe


def _fold(w_pitch, b_pitch, w_pos, b_pos, emb_beats, emb_singer, w_out, b_out):
    f64 = np.float64
    W1 = np.asarray(w_out[:D], f64)
    W2 = np.asarray(w_out[D:], f64)
    WposW1 = np.asarray(w_pos, f64) @ W1
    Wc = (W1 + WposW1).astype(np.float32)
    v = (np.asarray(w_pitch[0], f64) @ W1).astype(np.float32)
    EbW = np.asarray(emb_beats, f64) @ W1
    dEb = (EbW[1] - EbW[0]).astype(np.float32)
    EsW = (np.asarray(emb_singer, f64) @ W2).astype(np.float32)
    cb = (np.asarray(b_pitch + b_pos, f64) @ W1 + EbW[0] + np.asarray(b_out, f64))
    pe = _positional_encoding(T, D)
    PEW = (np.asarray(pe, f64) @ WposW1 + cb[None, :]).astype(np.float32)
    return Wc, v, dEb, EsW, PEW


def _tile_blocks(idx):
    """Per-core needed EW-block sets: blocks[c][k] = sorted j list for that
    core's output tile k (j indexes the 4 per-batch phone blocks)."""
    blocks = []
    for c in range(NCORES):
        per_tile = []
        for tt in range(TT):
            for b in range(BPC):
                row = idx[c * BPC + b, tt * 128:(tt + 1) * 128]
                per_tile.append(sorted(set(int(x) // 128 for x in (row[0], row[-1]))))
        blocks.append(per_tile)
    return blocks


_CACHE = {}


def kernel(encoder_out, align_phone, text_phone, pitch, beats, singer_vec,
           w_pitch, b_pitch, w_pos, b_pos, emb_beats, emb_singer, w_out, b_out):
    encoder_out = np.ascontiguousarray(np.asarray(encoder_out, np.float32))
    pitch = np.asarray(pitch, np.float32)[..., 0]          # [B,T]
    beats_f = np.asarray(beats, np.int64)[..., 0].astype(np.float32)
    sv = np.asarray(singer_vec, np.int64)[..., 0].astype(np.int64)  # [B,T]

    idx = _host_scan_idx(align_phone, text_phone)          # [B,T] int32
    Wc, v, dEb, EsW, PEW = _fold(
        np.asarray(w_pitch, np.float32), np.asarray(b_pitch, np.float32),
        np.asarray(w_pos, np.float32), np.asarray(b_pos, np.float32),
        np.asarray(emb_beats, np.float32), np.asarray(emb_singer, np.float32),
        np.asarray(w_out, np.float32), np.asarray(b_out, np.float32))

    esb = np.zeros((128, D), np.float32)
    esb[:S] = EsW
    esb[100] = v
    esb[101] = dEb

    # Baked structure: per tile, union over cores of needed EW blocks.
    blocks = _tile_blocks(idx)
    tile_segs = []
    seg_cols = {}                     # (k, j) -> oh1 column block
    col = 0
    for k in range(NT):
        b = k % BPC
        union_j = sorted(set(j for c in range(NCORES) for j in blocks[c][k]))
        segs = []
        for j in union_j:
            seg_cols[(k, j)] = col
            segs.append((b * NEW + j, col))
            col += 1
        tile_segs.append(segs)
    nseg = col

    tpos = np.arange(128)
    in_maps = []
    for c in range(NCORES):
        b0 = c * BPC
        sl = slice(b0, b0 + BPC)
        encT = np.ascontiguousarray(
            encoder_out[sl].transpose(0, 2, 1).reshape(BPC * PH, D))
        oh1 = np.zeros((128, nseg * 128), np.float32)
        ohsp = np.zeros((128, NT * 128), np.float32)
        for k in range(NT):
            tt, b = k // BPC, k % BPC
            t0 = tt * 128
            idxs = idx[b0 + b, t0:t0 + 128].astype(np.int64)
            for j in blocks[c][k]:
                local = idxs - j * 128
                m = (local >= 0) & (local < 128)
                blk = oh1[:, seg_cols[(k, j)] * 128:(seg_cols[(k, j)] + 1) * 128]
                blk[local[m], tpos[m]] = 1.0
            blk = ohsp[:, k * 128:(k + 1) * 128]
            blk[sv[b0 + b, t0:t0 + 128], tpos] = 1.0
            blk[100, :] = pitch[b0 + b, t0:t0 + 128]
            blk[101, :] = beats_f[b0 + b, t0:t0 + 128]
        in_maps.append({
            "encT": encT.astype(BF16_NP),
            "wc": Wc.astype(BF16_NP),
            "pew": PEW.astype(BF16_NP),
            "esb": esb.astype(BF16_NP),
            "iden": np.eye(128, dtype=BF16_NP),
            "oh1": oh1.astype(BF16_NP),
            "ohsp": ohsp.astype(BF16_NP),
        })

    seg_key = tuple(tuple(s) for s in tile_segs)
    if _CACHE.get("seg_key") != seg_key:
        _CACHE["nc"] = build_program(tile_segs, nseg)
        _CACHE["seg_key"] = seg_key
        _CACHE["segs"] = (tile_segs, nseg)
    nc = _CACHE["nc"]

    _CACHE["last_in_maps"] = in_maps
    res = run_bass_kernel_spmd(nc, in_maps, core_ids=list(range(NCORES)))
    out = np.empty((B, T, D), np.float32)
    for c in range(NCORES):
        out[c * BPC:(c + 1) * BPC] = np.asarray(
            res.results[c]["out"], np.float32).reshape(BPC, T, D)
    return out


# revision 10
# speedup vs baseline: 380.9287x; 303.4702x over previous
"""Trainium2 Bass kernel for nn_Encoder_Postnet_combine (B=16,T=4096,P=512,D=512,S=100).

Math (algebraically folded from the reference):
  idx[b,t]   : sequential aligner scan (host, tiny integer recurrence)
  W1 = w_out[:D]; W2 = w_out[D:]
  Wc  = (I + w_pos) @ W1
  EW  = encoder_out @ Wc                       (device GEMM, per batch)
  v   = w_pitch[0] @ W1
  dEb = (emb_beats[1]-emb_beats[0]) @ W1
  EsW = emb_singer @ W2
  PEW = pe @ (w_pos @ W1) + (b_pitch+b_pos+emb_beats[0]) @ W1 + b_out
  out = leaky( EW[b,idx] + EsW[sv] + PEW[t] + pitch*v + beats*dEb , 0.01)

Device mapping (bf16 matmul inputs, fp32 PSUM accumulation):
  idx is monotone nondecreasing (steps of <=1), so each 128-frame output
  tile reads a <=128-row window of EW spanning <=2 of the 4 per-batch EW
  SBUF tiles.  The gather is a one-hot matmul accumulated in PSUM:
    ps  = sum_j oh1_j.T @ EW_tile_j   (aligner gather, per-tile j set baked)
        + ohsp.T @ esb                (singer one-hot rows 0..99, row 100 =
                                       pitch values, row 101 = beats values;
                                       esb rows = [EsW; v; dEb])
        + I.T @ pew_tile              (positional table add)
  out = ACT lrelu(ps)  -> DMA store.
The per-tile segment structure (union across cores, SPMD) is baked into the
program; one-hot contents ship as runtime tensors (zero blocks where a core
does not need a segment).

Sharding: data-parallel over batch, 2 batches per core on 8 cores.
"""
import numpy as np
import ml_dtypes

import concourse.bass as bass
import concourse.mybir as mybir
import concourse.tile as tile
from concourse.vector_clock import ScopedClock
from concourse.bass_utils import run_bass_kernel_spmd

F32 = mybir.dt.float32
BF16 = mybir.dt.bfloat16
BF16_NP = ml_dtypes.bfloat16

# Device-side output dtype: bf16 halves the dominant HBM store traffic; the
# host upcasts to fp32.  Worst-case added error ~0.4% of |out| — well inside
# the 2e-2 rel tolerance.
OUT_DT = BF16
OUT_NP = BF16_NP

# One-hot operand dtype (values are exactly representable 0/1 plus bf16-ish
# pitch rows; fp8e4 halves their HBM traffic if mixed-dtype matmul works).
OH_DT = BF16
OH_NP = BF16_NP

# How PEW[t] gets added: "mm" = identity matmul into PSUM (tensor engine),
# "dve" = DVE tensor_tensor add PSUM+bf16, "hybrid" = alternate per tile.
PEW_MODE = "mm"

B, T, PH, D, S = 16, 4096, 512, 512, 100
NCORES = 8
BPC = B // NCORES          # batches per core
TT = T // 128              # 32 t-tiles per batch
NT = BPC * TT              # 64 tiles per core
NEW = PH // 128            # 4 EW tiles per batch

# ---------------------------------------------------------------------------
# Workarounds for this walrus build: at most ONE sync wait per instruction
# (EventSemaphore: 2).


def _split_drain_and_barrier(self, tick_clock, wait_clock):
    nc = self.nc
    probe = nc.sync.nop()
    wait_clock.add_sem_waits(probe.ins, ScopedClock({None: tick_clock.global_clock}))
    si = probe.ins.sync_info
    if si is not None and si.on_wait and len(si.on_wait) > 1:
        waits = list(si.on_wait)
        si.on_wait = waits[:1]
        for w in waits[1:]:
            extra = nc.sync.nop()
            extra.ins.sync_info = mybir.SyncInfo(on_wait=[w], on_update=[])
    nc.sync.drain()
    nc.all_engine_barrier()
    assert self.sems is not None
    popped = nc._tile_sem_poison_stack.pop()
    assert popped is self._sem_poison
    nc.clear_and_free_semaphores(list(self.sems.allocated().values()))
    nc.all_engine_barrier()


tile.TileContext._drain_and_barrier = _split_drain_and_barrier


def _split_multi_waits(nc):
    counter = [0]

    def fresh_nop(engine, wait):
        counter[0] += 1
        nop = mybir.InstNoOp(name=f"waitsplit_{counter[0]}", ins=[], outs=[])
        nop.engine = engine
        nop.sync_info = mybir.SyncInfo(on_wait=[wait], on_update=[])
        return nop

    for fn in nc.m.functions:
        for blk in fn.blocks:
            new_insts = []
            for inst in blk.instructions:
                si = inst.sync_info
                limit = 2 if isinstance(inst, mybir.InstEventSemaphore) else 1
                if si is not None and si.on_wait and len(si.on_wait) > limit:
                    waits = list(si.on_wait)
                    for w in waits[:-limit]:
                        new_insts.append(fresh_nop(inst.engine, w))
                    si.on_wait = waits[-limit:]
                new_insts.append(inst)
            blk.instructions = new_insts


# ---------------------------------------------------------------------------
# Device program.  `tile_segs` is a length-NT list; tile_segs[k] is the list
# of (ew_tile_index, oh1_col_block) pairs baked for that output tile.


def build_program(tile_segs, nseg, repeat=1, use_loop=False):
    nc = bass.Bass()
    encT = nc.declare_dram_parameter("encT", [BPC * PH, D], BF16, isOutput=False)
    wc = nc.declare_dram_parameter("wc", [D, D], BF16, isOutput=False)
    pew = nc.declare_dram_parameter("pew", [T, D], BF16, isOutput=False)
    esb = nc.declare_dram_parameter("esb", [128, D], BF16, isOutput=False)
    iden = nc.declare_dram_parameter("iden", [128, 128], OH_DT, isOutput=False)
    oh1 = nc.declare_dram_parameter("oh1", [128, nseg * 128], OH_DT, isOutput=False)
    ohsp = nc.declare_dram_parameter("ohsp", [128, NT * 128], OH_DT, isOutput=False)
    out = nc.declare_dram_parameter("out", [BPC * T, D], OUT_DT, isOutput=True)

    with tile.TileContext(nc) as tc:
        with (
            tc.tile_pool(name="const", bufs=1) as cpool,
            tc.tile_pool(name="pew", bufs=4) as ppool,
            tc.tile_pool(name="outp", bufs=6) as opool,
            tc.tile_pool(name="psA", bufs=2, space="PSUM") as psumA,
            tc.tile_pool(name="psB", bufs=6, space="PSUM") as psumB,
        ):
            def body(_=None):
                # --- small/const inputs (phase-A deps first on the ring) ---
                wc_sb = []
                for ki in range(4):
                    w_t = cpool.tile([128, D], BF16, tag=f"wc{ki}")
                    nc.sync.dma_start(out=w_t[:], in_=wc[ki * 128:(ki + 1) * 128, :])
                    wc_sb.append(w_t)
                encT_sb = []
                for j in range(4 * BPC):
                    e_t = cpool.tile([128, D], BF16, tag=f"encT{j}")
                    nc.sync.dma_start(out=e_t[:], in_=encT[j * 128:(j + 1) * 128, :])
                    encT_sb.append(e_t)
                esb_sb = cpool.tile([128, D], BF16, tag="esb")
                nc.sync.dma_start(out=esb_sb[:], in_=esb[:])
                id_sb = cpool.tile([128, 128], OH_DT, tag="iden")
                nc.sync.dma_start(out=id_sb[:], in_=iden[:])
                oh1_sb = cpool.tile([128, nseg * 128], OH_DT, tag="oh1")
                nc.sync.dma_start(out=oh1_sb[:], in_=oh1[:])
                ohsp_sb = cpool.tile([128, NT * 128], OH_DT, tag="ohsp")
                nc.sync.dma_start(out=ohsp_sb[:], in_=ohsp[:])

                # --- phase A: EW = E @ Wc (per batch), cast to bf16 in SBUF ---
                ew_sb = []
                for b in range(BPC):
                    for mm in range(4):
                        ps = psumA.tile([128, D], F32, tag="ps_ew")
                        for ki in range(4):
                            nc.tensor.matmul(
                                out=ps[:],
                                lhsT=encT_sb[b * 4 + ki][:, mm * 128:(mm + 1) * 128],
                                rhs=wc_sb[ki][:],
                                start=(ki == 0),
                                stop=(ki == 3),
                            )
                        ew_t = cpool.tile([128, D], BF16, tag=f"ew{b}_{mm}")
                        nc.vector.tensor_copy(out=ew_t[:], in_=ps[:])
                        ew_sb.append(ew_t)

                # --- phase B: one-hot gathers + PEW, all PSUM-accumulated ---
                for tt in range(TT):
                    pew_t = ppool.tile([128, D], BF16, tag="pew_t")
                    nc.sync.dma_start(out=pew_t[:], in_=pew[tt * 128:(tt + 1) * 128, :])
                    for b in range(BPC):
                        k = tt * BPC + b
                        segs = tile_segs[k]
                        ps = psumB.tile([128, D], F32, tag="ps_b")
                        for si, (ew_i, col) in enumerate(segs):
                            nc.tensor.matmul(
                                out=ps[:],
                                lhsT=oh1_sb[:, col * 128:(col + 1) * 128],
                                rhs=ew_sb[ew_i][:],
                                start=(si == 0),
                                stop=False,
                            )
                        pew_on_mm = (PEW_MODE == "mm" or
                                     (PEW_MODE == "hybrid" and k % 2 == 0))
                        nc.tensor.matmul(
                            out=ps[:],
                            lhsT=ohsp_sb[:, k * 128:(k + 1) * 128],
                            rhs=esb_sb[:],
                            start=False,
                            stop=not pew_on_mm,
                        )
                        pew_on_mm = (PEW_MODE == "mm" or
                                     (PEW_MODE == "hybrid" and k % 2 == 0))
                        if pew_on_mm:
                            nc.tensor.matmul(
                                out=ps[:],
                                lhsT=id_sb[:],
                                rhs=pew_t[:],
                                start=False,
                                stop=True,
                            )
                            act_in = ps
                        else:
                            s_t = opool.tile([128, D], F32, tag="s_t")
                            nc.vector.tensor_tensor(
                                out=s_t[:], in0=ps[:], in1=pew_t[:],
                                op=mybir.AluOpType.add)
                            act_in = s_t
                        o_t = opool.tile([128, D], OUT_DT, tag="o_t")
                        nc.scalar.activation(out=o_t[:], in_=act_in[:],
                                             func=mybir.ActivationFunctionType.Lrelu,
                                             alpha=0.01)
                        r0 = b * T + tt * 128
                        nc.scalar.dma_start(out=out[r0:r0 + 128, :], in_=o_t[:])

            if use_loop:
                with tc.For_i(0, repeat, 1) as _i:
                    body()
            else:
                for _ in range(repeat):
                    body()

    _split_multi_waits(nc)
    return nc


# ---------------------------------------------------------------------------
# Host side


def _host_scan_idx(align, text):
    align = np.asarray(align, dtype=np.int64)
    text = np.asarray(text, dtype=np.int64)
    Bn, Tn = align.shape
    Pn = text.shape[1]
    idx = np.zeros((Bn, Tn), dtype=np.int32)
    ind = np.zeros(Bn, dtype=np.int64)
    rows = np.arange(Bn)
    cur = text[rows, ind]
    for t in range(1, Tn):
        a = align[:, t]
        stay = a == cur
        ind = np.where(stay, ind, np.minimum(ind + 1, Pn - 1))
        cur = np.where(stay, cur, text[rows, ind])
        idx[:, t] = ind
    return idx


def _positional_encoding(length, d_model):
    pos = np.arange(length, dtype=np.float32)[:, None]
    div = np.exp(np.arange(0, d_model, 2, dtype=np.float32)
                 * (-np.log(10000.0) / d_model))
    pe = np.zeros((length, d_model), np.float32)
    pe[:, 0::2] = np.sin(pos * div)
    pe[:, 1::2] = np.cos(pos * div)
    return pe


def _fold(w_pitch, b_pitch, w_pos, b_pos, emb_beats, emb_singer, w_out, b_out):
    f64 = np.float64
    W1 = np.asarray(w_out[:D], f64)
    W2 = np.asarray(w_out[D:], f64)
    WposW1 = np.asarray(w_pos, f64) @ W1
    Wc = (W1 + WposW1).astype(np.float32)
    v = (np.asarray(w_pitch[0], f64) @ W1).astype(np.float32)
    EbW = np.asarray(emb_beats, f64) @ W1
    dEb = (EbW[1] - EbW[0]).astype(np.float32)
    EsW = (np.asarray(emb_singer, f64) @ W2).astype(np.float32)
    cb = (np.asarray(b_pitch + b_pos, f64) @ W1 + EbW[0] + np.asarray(b_out, f64))
    pe = _positional_encoding(T, D)
    PEW = (np.asarray(pe, f64) @ WposW1 + cb[None, :]).astype(np.float32)
    return Wc, v, dEb, EsW, PEW


def _tile_blocks(idx):
    """Per-core needed EW-block sets: blocks[c][k] = sorted j list for that
    core's output tile k (j indexes the 4 per-batch phone blocks)."""
    blocks = []
    for c in range(NCORES):
        per_tile = []
        for tt in range(TT):
            for b in range(BPC):
                row = idx[c * BPC + b, tt * 128:(tt + 1) * 128]
                per_tile.append(sorted(set(int(x) // 128 for x in (row[0], row[-1]))))
        blocks.append(per_tile)
    return blocks


_CACHE = {}


def kernel(encoder_out, align_phone, text_phone, pitch, beats, singer_vec,
           w_pitch, b_pitch, w_pos, b_pos, emb_beats, emb_singer, w_out, b_out):
    encoder_out = np.ascontiguousarray(np.asarray(encoder_out, np.float32))
    pitch = np.asarray(pitch, np.float32)[..., 0]          # [B,T]
    beats_f = np.asarray(beats, np.int64)[..., 0].astype(np.float32)
    sv = np.asarray(singer_vec, np.int64)[..., 0].astype(np.int64)  # [B,T]

    idx = _host_scan_idx(align_phone, text_phone)          # [B,T] int32
    Wc, v, dEb, EsW, PEW = _fold(
        np.asarray(w_pitch, np.float32), np.asarray(b_pitch, np.float32),
        np.asarray(w_pos, np.float32), np.asarray(b_pos, np.float32),
        np.asarray(emb_beats, np.float32), np.asarray(emb_singer, np.float32),
        np.asarray(w_out, np.float32), np.asarray(b_out, np.float32))

    esb = np.zeros((128, D), np.float32)
    esb[:S] = EsW
    esb[100] = v
    esb[101] = dEb

    # Baked structure: per tile, union over cores of needed EW blocks.
    blocks = _tile_blocks(idx)
    tile_segs = []
    seg_cols = {}                     # (k, j) -> oh1 column block
    col = 0
    for k in range(NT):
        b = k % BPC
        union_j = sorted(set(j for c in range(NCORES) for j in blocks[c][k]))
        segs = []
        for j in union_j:
            seg_cols[(k, j)] = col
            segs.append((b * NEW + j, col))
            col += 1
        tile_segs.append(segs)
    nseg = col

    tpos = np.arange(128)
    in_maps = []
    for c in range(NCORES):
        b0 = c * BPC
        sl = slice(b0, b0 + BPC)
        encT = np.ascontiguousarray(
            encoder_out[sl].transpose(0, 2, 1).reshape(BPC * PH, D))
        oh1 = np.zeros((128, nseg * 128), np.float32)
        ohsp = np.zeros((128, NT * 128), np.float32)
        for k in range(NT):
            tt, b = k // BPC, k % BPC
            t0 = tt * 128
            idxs = idx[b0 + b, t0:t0 + 128].astype(np.int64)
            for j in blocks[c][k]:
                local = idxs - j * 128
                m = (local >= 0) & (local < 128)
                blk = oh1[:, seg_cols[(k, j)] * 128:(seg_cols[(k, j)] + 1) * 128]
                blk[local[m], tpos[m]] = 1.0
            blk = ohsp[:, k * 128:(k + 1) * 128]
            blk[sv[b0 + b, t0:t0 + 128], tpos] = 1.0
            blk[100, :] = pitch[b0 + b, t0:t0 + 128]
            blk[101, :] = beats_f[b0 + b, t0:t0 + 128]
        in_maps.append({
            "encT": encT.astype(BF16_NP),
            "wc": Wc.astype(BF16_NP),
            "pew": PEW.astype(BF16_NP),
            "esb": esb.astype(BF16_NP),
            "iden": np.eye(128, dtype=OH_NP),
            "oh1": oh1.astype(OH_NP),
            "ohsp": ohsp.astype(OH_NP),
        })

    seg_key = tuple(tuple(s) for s in tile_segs)
    if _CACHE.get("seg_key") != seg_key:
        _CACHE["nc"] = build_program(tile_segs, nseg)
        _CACHE["seg_key"] = seg_key
        _CACHE["segs"] = (tile_segs, nseg)
    nc = _CACHE["nc"]

    _CACHE["last_in_maps"] = in_maps
    res = run_bass_kernel_spmd(nc, in_maps, core_ids=list(range(NCORES)))
    out = np.empty((B, T, D), np.float32)
    for c in range(NCORES):
        out[c * BPC:(c + 1) * BPC] = np.asarray(
            res.results[c]["out"], np.float32).reshape(BPC, T, D)
    return out


# revision 13
# speedup vs baseline: 407.6657x; 1.0702x over previous
"""Trainium2 Bass kernel for nn_Encoder_Postnet_combine (B=16,T=4096,P=512,D=512,S=100).

Math (algebraically folded from the reference):
  idx[b,t]   : sequential aligner scan (host, tiny integer recurrence)
  W1 = w_out[:D]; W2 = w_out[D:]
  Wc  = (I + w_pos) @ W1
  EW  = encoder_out @ Wc                       (device GEMM, per batch)
  v   = w_pitch[0] @ W1
  dEb = (emb_beats[1]-emb_beats[0]) @ W1
  EsW = emb_singer @ W2
  PEW = pe @ (w_pos @ W1) + (b_pitch+b_pos+emb_beats[0]) @ W1 + b_out
  out = leaky( EW[b,idx] + EsW[sv] + PEW[t] + pitch*v + beats*dEb , 0.01)

Device mapping (bf16/fp8 matmul inputs, fp32 PSUM accumulation):
  idx is monotone nondecreasing (steps of <=1), so each 128-frame output
  tile reads a <=128-row window of EW spanning <=2 of the 4 per-batch EW
  SBUF tiles.  The gather is a one-hot matmul accumulated in PSUM:
    ps  = sum_j oh1_j.T @ EW_tile_j   (aligner gather, per-tile j set baked)
        + ohsp.T @ esb                (singer one-hot rows 0..99, row 100 =
                                       pitch values, row 101 = beats values;
                                       esb rows = [EsW; v; dEb])
        + I.T @ pew_tile              (positional table add)
  out = ACT lrelu(ps)  -> DMA store (bf16, host upcasts to fp32).
Both 128-frame tiles of a tt (b=0,1) share one [128,1024] PSUM pair, one
ACT lrelu and one 3D-strided store.  The per-tile segment structure (union
across cores, SPMD) is baked into the program; one-hot contents ship as
runtime tensors (zero blocks where a core does not need a segment).

Sharding: data-parallel over batch, 2 batches per core on 8 cores.
"""
import numpy as np
import ml_dtypes

import concourse.bass as bass
import concourse.mybir as mybir
import concourse.tile as tile
from concourse.vector_clock import ScopedClock
from concourse.bass_utils import run_bass_kernel_spmd

F32 = mybir.dt.float32
BF16 = mybir.dt.bfloat16
BF16_NP = ml_dtypes.bfloat16

# Device-side output dtype: bf16 halves the dominant HBM store traffic; the
# host upcasts to fp32.  Worst-case added error ~0.4% of |out| — well inside
# the 2e-2 rel tolerance.
OUT_DT = BF16
OUT_NP = BF16_NP

# One-hot operand dtype.  NOTE: fp8 lhsT with bf16 rhs compiles but is
# fatal on hardware (NRT_EXEC_UNIT_UNRECOVERABLE) — matmul operands must
# share a dtype family here, so these stay bf16.
OH_DT = BF16
OH_NP = BF16_NP

# How PEW[t] gets added: "mm" = identity matmul into PSUM (tensor engine),
# "dve" = DVE tensor_tensor add PSUM+bf16, "hybrid" = alternate per tile.
PEW_MODE = "mm"

OH_CHUNKS = 4              # split the big one-hot loads for earlier phase B

B, T, PH, D, S = 16, 4096, 512, 512, 100
NCORES = 8
BPC = B // NCORES          # batches per core
TT = T // 128              # 32 t-tiles per batch
NT = BPC * TT              # 64 tiles per core
NEW = PH // 128            # 4 EW tiles per batch

# ---------------------------------------------------------------------------
# Workarounds for this walrus build: at most ONE sync wait per instruction
# (EventSemaphore: 2).


def _split_drain_and_barrier(self, tick_clock, wait_clock):
    nc = self.nc
    probe = nc.sync.nop()
    wait_clock.add_sem_waits(probe.ins, ScopedClock({None: tick_clock.global_clock}))
    si = probe.ins.sync_info
    if si is not None and si.on_wait and len(si.on_wait) > 1:
        waits = list(si.on_wait)
        si.on_wait = waits[:1]
        for w in waits[1:]:
            extra = nc.sync.nop()
            extra.ins.sync_info = mybir.SyncInfo(on_wait=[w], on_update=[])
    nc.sync.drain()
    nc.all_engine_barrier()
    assert self.sems is not None
    popped = nc._tile_sem_poison_stack.pop()
    assert popped is self._sem_poison
    nc.clear_and_free_semaphores(list(self.sems.allocated().values()))
    nc.all_engine_barrier()


tile.TileContext._drain_and_barrier = _split_drain_and_barrier


def _split_multi_waits(nc):
    counter = [0]

    def fresh_nop(engine, wait):
        counter[0] += 1
        nop = mybir.InstNoOp(name=f"waitsplit_{counter[0]}", ins=[], outs=[])
        nop.engine = engine
        nop.sync_info = mybir.SyncInfo(on_wait=[wait], on_update=[])
        return nop

    for fn in nc.m.functions:
        for blk in fn.blocks:
            new_insts = []
            for inst in blk.instructions:
                si = inst.sync_info
                limit = 2 if isinstance(inst, mybir.InstEventSemaphore) else 1
                if si is not None and si.on_wait and len(si.on_wait) > limit:
                    waits = list(si.on_wait)
                    for w in waits[:-limit]:
                        new_insts.append(fresh_nop(inst.engine, w))
                    si.on_wait = waits[-limit:]
                new_insts.append(inst)
            blk.instructions = new_insts


# ---------------------------------------------------------------------------
# Device program.  `tile_segs` is a length-NT list; tile_segs[k] is the list
# of (ew_tile_index, oh1_col_block) pairs baked for that output tile.


def build_program(tile_segs, nseg, repeat=1, use_loop=False):
    nc = bass.Bass()
    encT = nc.declare_dram_parameter("encT", [BPC * PH, D], BF16, isOutput=False)
    wc = nc.declare_dram_parameter("wc", [D, D], BF16, isOutput=False)
    pew = nc.declare_dram_parameter("pew", [T, D], BF16, isOutput=False)
    esb = nc.declare_dram_parameter("esb", [128, D], BF16, isOutput=False)
    iden = nc.declare_dram_parameter("iden", [128, 128], OH_DT, isOutput=False)
    oh1 = nc.declare_dram_parameter("oh1", [128, nseg * 128], OH_DT, isOutput=False)
    ohsp = nc.declare_dram_parameter("ohsp", [128, NT * 128], OH_DT, isOutput=False)
    out = nc.declare_dram_parameter("out", [BPC, TT, 128, D], OUT_DT, isOutput=True)

    with tile.TileContext(nc) as tc:
        with (
            tc.tile_pool(name="const", bufs=1) as cpool,
            tc.tile_pool(name="pew", bufs=4) as ppool,
            tc.tile_pool(name="outp", bufs=4) as opool,
            tc.tile_pool(name="psA", bufs=2, space="PSUM") as psumA,
            tc.tile_pool(name="psB", bufs=3, space="PSUM") as psumB,
        ):
            def body(_=None):
                # --- small/const inputs (phase-A deps first on the ring) ---
                wc_sb = []
                for ki in range(4):
                    w_t = cpool.tile([128, D], BF16, tag=f"wc{ki}")
                    nc.sync.dma_start(out=w_t[:], in_=wc[ki * 128:(ki + 1) * 128, :])
                    wc_sb.append(w_t)
                encT_sb = []
                for j in range(4 * BPC):
                    e_t = cpool.tile([128, D], BF16, tag=f"encT{j}")
                    nc.sync.dma_start(out=e_t[:], in_=encT[j * 128:(j + 1) * 128, :])
                    encT_sb.append(e_t)
                esb_sb = cpool.tile([128, D], BF16, tag="esb")
                nc.sync.dma_start(out=esb_sb[:], in_=esb[:])
                id_sb = cpool.tile([128, 128], OH_DT, tag="iden")
                nc.sync.dma_start(out=id_sb[:], in_=iden[:])
                oh1_sb = cpool.tile([128, nseg * 128], OH_DT, tag="oh1")
                ncol1 = nseg * 128
                for ch in range(OH_CHUNKS):
                    c0 = (ncol1 * ch // OH_CHUNKS) // 128 * 128
                    c1 = (ncol1 * (ch + 1) // OH_CHUNKS) // 128 * 128
                    if ch == OH_CHUNKS - 1:
                        c1 = ncol1
                    if c1 > c0:
                        nc.sync.dma_start(out=oh1_sb[:, c0:c1], in_=oh1[:, c0:c1])
                ohsp_sb = cpool.tile([128, NT * 128], OH_DT, tag="ohsp")
                ncol2 = NT * 128
                for ch in range(OH_CHUNKS):
                    c0 = (ncol2 * ch // OH_CHUNKS) // 128 * 128
                    c1 = (ncol2 * (ch + 1) // OH_CHUNKS) // 128 * 128
                    if ch == OH_CHUNKS - 1:
                        c1 = ncol2
                    if c1 > c0:
                        nc.sync.dma_start(out=ohsp_sb[:, c0:c1], in_=ohsp[:, c0:c1])

                # --- phase A: EW = E @ Wc (per batch), cast to bf16 in SBUF ---
                ew_sb = []
                for b in range(BPC):
                    for mm in range(4):
                        ps = psumA.tile([128, D], F32, tag="ps_ew")
                        for ki in range(4):
                            nc.tensor.matmul(
                                out=ps[:],
                                lhsT=encT_sb[b * 4 + ki][:, mm * 128:(mm + 1) * 128],
                                rhs=wc_sb[ki][:],
                                start=(ki == 0),
                                stop=(ki == 3),
                            )
                        ew_t = cpool.tile([128, D], BF16, tag=f"ew{b}_{mm}")
                        nc.vector.tensor_copy(out=ew_t[:], in_=ps[:])
                        ew_sb.append(ew_t)

                # --- phase B: one-hot gathers + PEW, all PSUM-accumulated;
                #     both b-halves of a tt share one [128,1024] PSUM pair ---
                for tt in range(TT):
                    pew_t = ppool.tile([128, D], BF16, tag="pew_t")
                    nc.sync.dma_start(out=pew_t[:], in_=pew[tt * 128:(tt + 1) * 128, :])
                    ps = psumB.tile([128, 2 * D], F32, tag="ps_b")
                    for b in range(BPC):
                        k = tt * BPC + b
                        segs = tile_segs[k]
                        half = ps[:, b * D:(b + 1) * D]
                        for si, (ew_i, col) in enumerate(segs):
                            nc.tensor.matmul(
                                out=half,
                                lhsT=oh1_sb[:, col * 128:(col + 1) * 128],
                                rhs=ew_sb[ew_i][:],
                                start=(si == 0),
                                stop=False,
                            )
                        pew_on_mm = (PEW_MODE == "mm" or
                                     (PEW_MODE == "hybrid" and k % 2 == 0))
                        nc.tensor.matmul(
                            out=half,
                            lhsT=ohsp_sb[:, k * 128:(k + 1) * 128],
                            rhs=esb_sb[:],
                            start=False,
                            stop=not pew_on_mm,
                        )
                        if pew_on_mm:
                            nc.tensor.matmul(
                                out=half,
                                lhsT=id_sb[:],
                                rhs=pew_t[:],
                                start=False,
                                stop=True,
                            )
                    o_t = opool.tile([128, 2 * D], OUT_DT, tag="o_t")
                    if PEW_MODE == "mm":
                        nc.scalar.activation(out=o_t[:], in_=ps[:],
                                             func=mybir.ActivationFunctionType.Lrelu,
                                             alpha=0.01)
                    else:
                        for b in range(BPC):
                            k = tt * BPC + b
                            sl = slice(b * D, (b + 1) * D)
                            if PEW_MODE == "mm" or (PEW_MODE == "hybrid"
                                                    and k % 2 == 0):
                                nc.scalar.activation(
                                    out=o_t[:, sl], in_=ps[:, sl],
                                    func=mybir.ActivationFunctionType.Lrelu,
                                    alpha=0.01)
                            else:
                                s_t = opool.tile([128, D], F32, tag="s_t")
                                nc.vector.tensor_tensor(
                                    out=s_t[:], in0=ps[:, sl], in1=pew_t[:],
                                    op=mybir.AluOpType.add)
                                nc.scalar.activation(
                                    out=o_t[:, sl], in_=s_t[:],
                                    func=mybir.ActivationFunctionType.Lrelu,
                                    alpha=0.01)
                    for b in range(BPC):
                        nc.scalar.dma_start(
                            out=out[b, tt],
                            in_=o_t[:, b * D:(b + 1) * D])

            if use_loop:
                with tc.For_i(0, repeat, 1) as _i:
                    body()
            else:
                for _ in range(repeat):
                    body()

    _split_multi_waits(nc)
    return nc


# ---------------------------------------------------------------------------
# Host side


def _host_scan_idx(align, text):
    align = np.asarray(align, dtype=np.int64)
    text = np.asarray(text, dtype=np.int64)
    Bn, Tn = align.shape
    Pn = text.shape[1]
    idx = np.zeros((Bn, Tn), dtype=np.int32)
    ind = np.zeros(Bn, dtype=np.int64)
    rows = np.arange(Bn)
    cur = text[rows, ind]
    for t in range(1, Tn):
        a = align[:, t]
        stay = a == cur
        ind = np.where(stay, ind, np.minimum(ind + 1, Pn - 1))
        cur = np.where(stay, cur, text[rows, ind])
        idx[:, t] = ind
    return idx


def _positional_encoding(length, d_model):
    pos = np.arange(length, dtype=np.float32)[:, None]
    div = np.exp(np.arange(0, d_model, 2, dtype=np.float32)
                 * (-np.log(10000.0) / d_model))
    pe = np.zeros((length, d_model), np.float32)
    pe[:, 0::2] = np.sin(pos * div)
    pe[:, 1::2] = np.cos(pos * div)
    return pe


def _fold(w_pitch, b_pitch, w_pos, b_pos, emb_beats, emb_singer, w_out, b_out):
    f64 = np.float64
    W1 = np.asarray(w_out[:D], f64)
    W2 = np.asarray(w_out[D:], f64)
    WposW1 = np.asarray(w_pos, f64) @ W1
    Wc = (W1 + WposW1).astype(np.float32)
    v = (np.asarray(w_pitch[0], f64) @ W1).astype(np.float32)
    EbW = np.asarray(emb_beats, f64) @ W1
    dEb = (EbW[1] - EbW[0]).astype(np.float32)
    EsW = (np.asarray(emb_singer, f64) @ W2).astype(np.float32)
    cb = (np.asarray(b_pitch + b_pos, f64) @ W1 + EbW[0] + np.asarray(b_out, f64))
    pe = _positional_encoding(T, D)
    PEW = (np.asarray(pe, f64) @ WposW1 + cb[None, :]).astype(np.float32)
    return Wc, v, dEb, EsW, PEW


def _tile_blocks(idx):
    """Per-core needed EW-block sets: blocks[c][k] = sorted j list for that
    core's output tile k (j indexes the 4 per-batch phone blocks)."""
    blocks = []
    for c in range(NCORES):
        per_tile = []
        for tt in range(TT):
            for b in range(BPC):
                row = idx[c * BPC + b, tt * 128:(tt + 1) * 128]
                per_tile.append(sorted(set(int(x) // 128 for x in (row[0], row[-1]))))
        blocks.append(per_tile)
    return blocks


_CACHE = {}


def kernel(encoder_out, align_phone, text_phone, pitch, beats, singer_vec,
           w_pitch, b_pitch, w_pos, b_pos, emb_beats, emb_singer, w_out, b_out):
    encoder_out = np.ascontiguousarray(np.asarray(encoder_out, np.float32))
    pitch = np.asarray(pitch, np.float32)[..., 0]          # [B,T]
    beats_f = np.asarray(beats, np.int64)[..., 0].astype(np.float32)
    sv = np.asarray(singer_vec, np.int64)[..., 0].astype(np.int64)  # [B,T]

    idx = _host_scan_idx(align_phone, text_phone)          # [B,T] int32
    Wc, v, dEb, EsW, PEW = _fold(
        np.asarray(w_pitch, np.float32), np.asarray(b_pitch, np.float32),
        np.asarray(w_pos, np.float32), np.asarray(b_pos, np.float32),
        np.asarray(emb_beats, np.float32), np.asarray(emb_singer, np.float32),
        np.asarray(w_out, np.float32), np.asarray(b_out, np.float32))

    esb = np.zeros((128, D), np.float32)
    esb[:S] = EsW
    esb[100] = v
    esb[101] = dEb

    # Baked structure: per tile, union over cores of needed EW blocks.
    blocks = _tile_blocks(idx)
    tile_segs = []
    seg_cols = {}                     # (k, j) -> oh1 column block
    col = 0
    for k in range(NT):
        b = k % BPC
        union_j = sorted(set(j for c in range(NCORES) for j in blocks[c][k]))
        segs = []
        for j in union_j:
            seg_cols[(k, j)] = col
            segs.append((b * NEW + j, col))
            col += 1
        tile_segs.append(segs)
    nseg = col

    tpos = np.arange(128)
    in_maps = []
    for c in range(NCORES):
        b0 = c * BPC
        sl = slice(b0, b0 + BPC)
        encT = np.ascontiguousarray(
            encoder_out[sl].transpose(0, 2, 1).reshape(BPC * PH, D))
        oh1 = np.zeros((128, nseg * 128), np.float32)
        ohsp = np.zeros((128, NT * 128), np.float32)
        for k in range(NT):
            tt, b = k // BPC, k % BPC
            t0 = tt * 128
            idxs = idx[b0 + b, t0:t0 + 128].astype(np.int64)
            for j in blocks[c][k]:
                local = idxs - j * 128
                m = (local >= 0) & (local < 128)
                blk = oh1[:, seg_cols[(k, j)] * 128:(seg_cols[(k, j)] + 1) * 128]
                blk[local[m], tpos[m]] = 1.0
            blk = ohsp[:, k * 128:(k + 1) * 128]
            blk[sv[b0 + b, t0:t0 + 128], tpos] = 1.0
            blk[100, :] = pitch[b0 + b, t0:t0 + 128]
            blk[101, :] = beats_f[b0 + b, t0:t0 + 128]
        in_maps.append({
            "encT": encT.astype(BF16_NP),
            "wc": Wc.astype(BF16_NP),
            "pew": PEW.astype(BF16_NP),
            "esb": esb.astype(BF16_NP),
            "iden": np.eye(128, dtype=OH_NP),
            "oh1": oh1.astype(OH_NP),
            "ohsp": ohsp.astype(OH_NP),
        })

    seg_key = tuple(tuple(s) for s in tile_segs)
    if _CACHE.get("seg_key") != seg_key:
        _CACHE["nc"] = build_program(tile_segs, nseg)
        _CACHE["seg_key"] = seg_key
        _CACHE["segs"] = (tile_segs, nseg)
    nc = _CACHE["nc"]

    _CACHE["last_in_maps"] = in_maps
    res = run_bass_kernel_spmd(nc, in_maps, core_ids=list(range(NCORES)))
    out = np.empty((B, T, D), np.float32)
    for c in range(NCORES):
        out[c * BPC:(c + 1) * BPC] = np.asarray(
            res.results[c]["out"], np.float32).reshape(BPC, T, D)
    return out
